# revision 1
# baseline (speedup 1.0000x reference)
"""GCN (2x GCNConv + MLP head) on 8 TRN2 NeuronCores via Bass/Tile.

Distribution (graph-parallel, per the node-sharding scheme):
  - nodes sharded by id across 8 cores (12500 each); weights replicated.
  - Phase A (replicated): h1l rows = (dinv*x) @ W1 for ALL nodes -> DRAM.
  - Conv edge phase (sharded by dst): for each core's in-edges,
    dma_gather 256B message rows by src id, then per-128-edge block a
    DVE-built one-hot S_dst and a PE matmul accumulate aggT[64,128] per
    dst tile in PSUM (exact f32); epilogue h1T = dinv*aggT + b1.
  - AllGather of h1T shards (bf16) = the halo exchange.
  - Phase C (replicated): h2l rows = h1 @ W2 for ALL nodes -> DRAM.
  - Conv2 edge phase -> h2T (f32, SBUF resident).
  - MLP head in transposed space; output row [1, shard].

Host preprocessing is structure-only (derived from edge_index): degrees,
edge blocking by (dst-tile, src-window), int16 gather indices. All cores
share one program: block structure is padded to the max across cores.
"""

import numpy as np
import ml_dtypes

import concourse.bass as bass
import concourse.bacc as bacc
import concourse.tile as tile
import concourse.mybir as mybir
from concourse.bass_utils import run_bass_kernel_spmd

F32 = mybir.dt.float32
BF16 = mybir.dt.bfloat16
I16 = mybir.dt.int16

NCORES = 8
WIN = 25088          # gather window rows (multiple of 128, < int16 max)
GAP = 128            # zero rows appended per window (pad-edge target)
WSTRIDE = WIN + GAP
TILE = 128           # dst tile size
CB = 8               # max 128-edge blocks per dma_gather (1024-idx HW limit)


# ----------------------------------------------------------------------------
# host-side preprocessing (numpy only)
# ----------------------------------------------------------------------------

def wrap16x8(a):
    """[n] int16 -> [128, n//16]: idx i at [i%16, i//16], replicated x8."""
    w = np.ascontiguousarray(np.transpose(a.reshape(-1, 16), (1, 0)))
    return np.ascontiguousarray(np.tile(w, (8, 1)))


def preprocess(n, edge_index):
    """Uniform cross-core edge plan.

    Returns (dinv, plan, cores) where plan holds the shared structure
    (chunks/blocks/flags) and cores[c] holds per-core staged index arrays.
    """
    src = edge_index[0].astype(np.int64)
    dst = edge_index[1].astype(np.int64)

    deg = np.bincount(dst, minlength=n).astype(np.float64) + 1.0
    dinv = (1.0 / np.sqrt(deg)).astype(np.float32)

    shard = n // NCORES
    assert shard * NCORES == n and shard % 2 == 0
    ntiles = (shard + TILE - 1) // TILE
    dpad = ntiles * TILE
    nwin = (n + WIN - 1) // WIN

    loops = np.arange(n, dtype=np.int64)
    src = np.concatenate([src, loops])
    dst = np.concatenate([dst, loops])

    # per-core edge lists grouped by (dst tile, src window)
    per_core = []
    counts = np.zeros((NCORES, ntiles, nwin), np.int64)
    for c in range(NCORES):
        base = c * shard
        m = (dst >= base) & (dst < base + shard)
        s, d = src[m], dst[m] - base
        t_id = d // TILE
        w_id = s // WIN
        order = np.lexsort((w_id, t_id))
        s, d, t_id, w_id = s[order], d[order], t_id[order], w_id[order]
        np.add.at(counts[c], (t_id, w_id), 1)
        per_core.append((s, d, t_id, w_id))

    nb = (counts.max(axis=0) + TILE - 1) // TILE      # [ntiles, nwin] blocks

    # shared chunk/block structure, tile-major
    chunks = []   # (window, n_blocks, tile)
    blocks = []   # (tile, start, stop)
    for t in range(ntiles):
        tile_blocks = int(nb[t].sum())
        done = 0
        for w in range(nwin):
            g = int(nb[t, w])
            b0 = 0
            while b0 < g:
                k = min(CB, g - b0)
                chunks.append((w, k, t))
                for j in range(k):
                    bi = done + b0 + j
                    blocks.append((t, bi == 0, bi == tile_blocks - 1))
                b0 += k
            done += g
    goff, boff = [], []
    g0 = b0_ = 0
    for (w, k, t) in chunks:
        goff.append(g0); boff.append(b0_)
        g0 += k * TILE // 16
        b0_ += k

    # per-core staged arrays
    cores = []
    for c in range(NCORES):
        s, d, t_id, w_id = per_core[c]
        gidx = np.full((b0_ * TILE,), WIN, np.int16)    # default: pad row
        dstl = np.zeros((b0_ * TILE,), np.float32)
        # locate each core group inside the shared layout
        key = t_id * nwin + w_id
        cuts = np.flatnonzero(np.diff(key)) + 1
        starts = np.concatenate([[0], cuts]) if len(s) else np.array([], np.int64)
        ends = np.concatenate([cuts, [len(s)]]) if len(s) else np.array([], np.int64)
        # block offset of group (t, w) in the shared layout
        grp_boff = np.zeros((ntiles, nwin), np.int64)
        acc = 0
        for t in range(ntiles):
            for w in range(nwin):
                grp_boff[t, w] = acc
                acc += nb[t, w]
        for a, b in zip(starts, ends):
            t = int(t_id[a]); w = int(w_id[a])
            o = grp_boff[t, w] * TILE
            cnt = b - a
            gidx[o:o + cnt] = (s[a:b] - w * WIN).astype(np.int16)
            dstl[o:o + cnt] = (d[a:b] - t * TILE).astype(np.float32)
        cores.append(dict(
            gidx=wrap16x8(gidx),
            dstl=np.ascontiguousarray(dstl.reshape(b0_, TILE).T),
            base=c * shard,
        ))

    plan = dict(chunks=chunks, blocks=blocks, goff=goff, boff=boff,
                ntiles=ntiles, dpad=dpad, shard=shard, nwin=nwin,
                gcols=g0, bcols=b0_)
    return dinv, plan, cores


# ----------------------------------------------------------------------------
# device program
# ----------------------------------------------------------------------------

def emit_conv_edges(nc, pool, ipool, psum, plan, hbuf, gidx_d, dstl_d, iota_t,
                    dinvrep_t, bias_t, out_cb, out_dtype):
    """One conv's edge aggregation. out_cb(tile_idx, ap_or_tile)."""
    agg = {"t": None}
    bi = 0
    for ci, (w, k, t) in enumerate(plan["chunks"]):
        go = plan["goff"][ci]
        bo = plan["boff"][ci]
        nidx = k * TILE
        it = ipool.tile([128, CB * TILE // 16], I16, tag="gidx")
        nc.sync.dma_start(it[:, :nidx // 16], gidx_d[:, go:go + nidx // 16])
        dl = ipool.tile([128, CB], F32, tag="dstl")
        nc.sync.dma_start(dl[:, :k], dstl_d[:, bo:bo + k])
        g = pool.tile([128, CB, 64], F32, tag="g")
        nc.gpsimd.dma_gather(
            g[:, :k, :],
            hbuf[w * WSTRIDE:(w + 1) * WSTRIDE, :],
            it[:, :nidx // 16],
            num_idxs=nidx, num_idxs_reg=nidx, elem_size=64,
        )
        s_t = pool.tile([128, CB, TILE], F32, tag="s")
        nc.vector.tensor_tensor(
            s_t[:, :k, :],
            iota_t[:].unsqueeze(1).broadcast_to([128, k, TILE]),
            dl[:, :k].unsqueeze(2).broadcast_to([128, k, TILE]),
            op=mybir.AluOpType.is_equal,
        )
        for j in range(k):
            t_, start, stop = plan["blocks"][bi]; bi += 1
            if start:
                agg["t"] = psum.tile([64, TILE], F32, tag="agg", name=f"agg_{bi}")
            nc.tensor.matmul(agg["t"][:], lhsT=g[:, j, :], rhs=s_t[:, j, :],
                             start=start, stop=stop)
            if stop:
                ag = agg["t"]
                e1 = pool.tile([64, TILE], F32, tag="ep1")
                nc.vector.tensor_tensor(
                    e1[:], ag[:],
                    dinvrep_t[:, t_ * TILE:(t_ + 1) * TILE],
                    op=mybir.AluOpType.mult)
                e2 = pool.tile([64, TILE], out_dtype, tag="ep2")
                nc.vector.tensor_tensor(
                    e2[:], e1[:], bias_t[:].broadcast_to([64, TILE]),
                    op=mybir.AluOpType.add)
                out_cb(t_, e2)


def build_program(meta, plan):
    n = meta["n"]
    npad = meta["npad"]
    nwin = npad // WIN
    hrows = nwin * WSTRIDE
    dpad = plan["dpad"]
    shard = plan["shard"]
    ntiles = plan["ntiles"]
    gcols = max(plan["gcols"], 16)
    bcols = max(plan["bcols"], 1)

    nc = bacc.Bacc("TRN2", target_bir_lowering=False, debug=False,
                   num_devices=NCORES)

    xt = nc.dram_tensor("xt", [128, npad], BF16, kind="ExternalInput")
    h1buf = nc.dram_tensor("h1buf", [hrows, 64], F32, kind="ExternalInput")
    h2buf = nc.dram_tensor("h2buf", [hrows, 64], F32, kind="ExternalInput")
    gidx_d = nc.dram_tensor("gidx", [128, gcols], I16, kind="ExternalInput")
    dstl_d = nc.dram_tensor("dstl", [128, bcols], F32, kind="ExternalInput")
    w1_d = nc.dram_tensor("w1", [128, 64], BF16, kind="ExternalInput")
    w2_d = nc.dram_tensor("w2", [64, 64], BF16, kind="ExternalInput")
    lw1_d = nc.dram_tensor("lw1", [64, 64], F32, kind="ExternalInput")
    lw2_d = nc.dram_tensor("lw2", [64, 32], F32, kind="ExternalInput")
    lw3_d = nc.dram_tensor("lw3", [32, 1], F32, kind="ExternalInput")
    b1_d = nc.dram_tensor("b1", [64, 1], F32, kind="ExternalInput")
    b2_d = nc.dram_tensor("b2", [64, 1], F32, kind="ExternalInput")
    lb1_d = nc.dram_tensor("lb1", [64, 1], F32, kind="ExternalInput")
    lb2_d = nc.dram_tensor("lb2", [32, 1], F32, kind="ExternalInput")
    lb3_d = nc.dram_tensor("lb3", [1, 1], F32, kind="ExternalInput")
    iota_d = nc.dram_tensor("iota", [128, TILE], F32, kind="ExternalInput")
    dinvrep_d = nc.dram_tensor("dinvrep", [64, dpad], F32, kind="ExternalInput")
    dinvc_d = nc.dram_tensor("dinvc", [128, NCORES * ntiles], F32,
                             kind="ExternalInput")
    out_d = nc.dram_tensor("out", [1, dpad], F32, kind="ExternalOutput")

    with tile.TileContext(nc) as tc:
        with (
            tc.tile_pool(name="const", bufs=1) as cpool,
            tc.tile_pool(name="work", bufs=6) as pool,
            tc.tile_pool(name="idx", bufs=6) as ipool,
            tc.tile_pool(name="xtp", bufs=4) as xtpool,
            tc.tile_pool(name="psag", bufs=2, space="PSUM") as psag,
            tc.tile_pool(name="psmm", bufs=4, space="PSUM") as psmm,
            tc.tile_pool(name="dram", bufs=1, space="DRAM") as dram,
        ):
            def load_const(dram_t, shape, dtype, tag):
                t = cpool.tile(shape, dtype, tag=tag)
                nc.sync.dma_start(t[:], dram_t[:])
                return t

            w1_t = load_const(w1_d, [128, 64], BF16, "w1")
            w2_t = load_const(w2_d, [64, 64], BF16, "w2")
            lw1_t = load_const(lw1_d, [64, 64], F32, "lw1")
            lw2_t = load_const(lw2_d, [64, 32], F32, "lw2")
            lw3_t = load_const(lw3_d, [32, 1], F32, "lw3")
            b1_t = load_const(b1_d, [64, 1], F32, "b1")
            b2_t = load_const(b2_d, [64, 1], F32, "b2")
            lb1_t = load_const(lb1_d, [64, 1], F32, "lb1")
            lb2_t = load_const(lb2_d, [32, 1], F32, "lb2")
            lb3_t = load_const(lb3_d, [1, 1], F32, "lb3")
            iota_t = load_const(iota_d, [128, TILE], F32, "iota")
            dinvrep_t = load_const(dinvrep_d, [64, dpad], F32, "dinvrep")
            dinvc_t = load_const(dinvc_d, [128, NCORES * ntiles], F32, "dinvc")

            # --- phase A ---
            for t in range(npad // TILE):
                st = xtpool.tile([128, TILE], BF16, tag="xt")
                nc.sync.dma_start(st[:], xt[:, t * TILE:(t + 1) * TILE])
                ps = psmm.tile([TILE, 64], F32, tag="mm")
                nc.tensor.matmul(ps[:], lhsT=st[:], rhs=w1_t[:],
                                 start=True, stop=True)
                sb = pool.tile([TILE, 64], F32, tag="arow")
                nc.vector.tensor_copy(sb[:], ps[:])
                w = (t * TILE) // WIN
                r = w * WSTRIDE + (t * TILE) % WIN
                nc.sync.dma_start(h1buf[r:r + TILE, :], sb[:])

            # --- conv1 edges -> h1T bf16 bounce ---
            h1t_bounce = dram.tile([64, dpad], BF16)
            ag_out = dram.tile([NCORES * 64, dpad], BF16, addr_space="Shared")

            def conv1_out(t_, e2):
                nc.sync.dma_start(h1t_bounce[:, t_ * TILE:(t_ + 1) * TILE], e2[:])

            emit_conv_edges(nc, pool, ipool, psag, plan, h1buf, gidx_d, dstl_d,
                            iota_t, dinvrep_t, b1_t, conv1_out, BF16)

            if dpad > shard:
                zt = pool.tile([64, dpad - shard], BF16, tag="zt")
                nc.vector.memset(zt[:], 0.0)
                nc.sync.dma_start(h1t_bounce[:, shard:], zt[:])

            nc.gpsimd.collective_compute(
                "AllGather", mybir.AluOpType.bypass,
                ins=[h1t_bounce[:].opt()],
                outs=[ag_out[:].opt()],
                replica_groups=[list(range(NCORES))],
            )

            # --- phase C: h2l rows for all nodes ---
            for c in range(NCORES):
                for t in range(ntiles):
                    n0 = c * shard + t * TILE
                    cnt = min(TILE, shard - t * TILE)
                    st = xtpool.tile([64, TILE], BF16, tag="ct")
                    nc.sync.dma_start(
                        st[:, :cnt],
                        ag_out[c * 64:(c + 1) * 64, t * TILE:t * TILE + cnt])
                    ps = psmm.tile([TILE, 64], F32, tag="mm")
                    nc.tensor.matmul(ps[:cnt, :], lhsT=st[:, :cnt], rhs=w2_t[:],
                                     start=True, stop=True)
                    sb = pool.tile([TILE, 64], F32, tag="crow")
                    nc.vector.tensor_tensor(
                        sb[:cnt, :], ps[:cnt, :],
                        dinvc_t[:cnt, c * ntiles + t:c * ntiles + t + 1]
                        .broadcast_to([cnt, 64]),
                        op=mybir.AluOpType.mult)
                    off = 0
                    while off < cnt:
                        nn = n0 + off
                        w = nn // WIN
                        take = min(cnt - off, (w + 1) * WIN - nn)
                        r = w * WSTRIDE + (nn % WIN)
                        nc.sync.dma_start(h2buf[r:r + take, :],
                                          sb[off:off + take, :])
                        off += take

            # --- conv2 edges -> h2T f32 in SBUF ---
            h2t_sb = cpool.tile([64, dpad], F32, tag="h2t")

            def conv2_out(t_, e2):
                nc.vector.tensor_copy(h2t_sb[:, t_ * TILE:(t_ + 1) * TILE],
                                      e2[:])

            emit_conv_edges(nc, pool, ipool, psag, plan, h2buf, gidx_d, dstl_d,
                            iota_t, dinvrep_t, b2_t, conv2_out, F32)

            # --- MLP head (transposed space) ---
            EC = 512
            for o in range(0, dpad, EC):
                w_ = min(EC, dpad - o)
                p1 = psmm.tile([64, EC], F32, tag="mm")
                nc.tensor.matmul(p1[:, :w_], lhsT=lw1_t[:],
                                 rhs=h2t_sb[:, o:o + w_], start=True, stop=True)
                z1 = pool.tile([64, EC], F32, tag="z1")
                nc.scalar.activation(z1[:, :w_], p1[:, :w_],
                                     mybir.ActivationFunctionType.Relu,
                                     bias=lb1_t[:])
                p2 = psmm.tile([32, EC], F32, tag="mm")
                nc.tensor.matmul(p2[:, :w_], lhsT=lw2_t[:], rhs=z1[:, :w_],
                                 start=True, stop=True)
                z2 = pool.tile([32, EC], F32, tag="z2")
                nc.scalar.activation(z2[:, :w_], p2[:, :w_],
                                     mybir.ActivationFunctionType.Relu,
                                     bias=lb2_t[:])
                p3 = psmm.tile([1, EC], F32, tag="mm")
                nc.tensor.matmul(p3[:, :w_], lhsT=lw3_t[:], rhs=z2[:, :w_],
                                 start=True, stop=True)
                z3 = pool.tile([1, EC], F32, tag="z3")
                nc.vector.tensor_tensor(z3[:, :w_], p3[:, :w_],
                                        lb3_t[:].broadcast_to([1, w_]),
                                        op=mybir.AluOpType.add)
                nc.sync.dma_start(out_d[:, o:o + w_], z3[:, :w_])

    nc.compile()
    return nc


# ----------------------------------------------------------------------------
# entry point
# ----------------------------------------------------------------------------

def kernel(x, edge_index, W1, b1, W2, b2, lw1, lb1, lw2, lb2, lw3, lb3,
           _want_trace=False):
    x = np.asarray(x, np.float32)
    edge_index = np.asarray(edge_index)
    n = x.shape[0]
    npad = ((n + WIN - 1) // WIN) * WIN
    nwin = npad // WIN
    hrows = nwin * WSTRIDE

    dinv, plan, cores = preprocess(n, edge_index)
    shard, dpad, ntiles = plan["shard"], plan["dpad"], plan["ntiles"]

    xt = np.zeros((128, npad), ml_dtypes.bfloat16)
    xt[:, :n] = (x * dinv[:, None]).T.astype(ml_dtypes.bfloat16)
    hz = np.zeros((hrows, 64), np.float32)
    iota = np.tile(np.arange(TILE, dtype=np.float32), (128, 1))

    dinvc = np.zeros((128, NCORES * ntiles), np.float32)
    for cc in range(NCORES):
        for t in range(ntiles):
            n0 = cc * shard + t * TILE
            cnt = min(TILE, (cc + 1) * shard - n0)
            dinvc[:cnt, cc * ntiles + t] = dinv[n0:n0 + cnt]

    in_maps = []
    for c in range(NCORES):
        dinvrep = np.zeros((64, dpad), np.float32)
        dinvrep[:, :shard] = dinv[c * shard:(c + 1) * shard][None, :]
        in_maps.append({
            "xt": xt, "h1buf": hz, "h2buf": hz,
            "gidx": cores[c]["gidx"], "dstl": cores[c]["dstl"],
            "w1": np.asarray(W1, np.float32).astype(ml_dtypes.bfloat16),
            "w2": np.asarray(W2, np.float32).astype(ml_dtypes.bfloat16),
            "lw1": np.ascontiguousarray(np.asarray(lw1, np.float32)),
            "lw2": np.ascontiguousarray(np.asarray(lw2, np.float32)),
            "lw3": np.ascontiguousarray(np.asarray(lw3, np.float32)),
            "b1": np.asarray(b1, np.float32).reshape(-1, 1),
            "b2": np.asarray(b2, np.float32).reshape(-1, 1),
            "lb1": np.asarray(lb1, np.float32).reshape(-1, 1),
            "lb2": np.asarray(lb2, np.float32).reshape(-1, 1),
            "lb3": np.asarray(lb3, np.float32).reshape(-1, 1),
            "iota": iota, "dinvrep": dinvrep, "dinvc": dinvc,
        })

    meta = {"n": n, "npad": npad}
    nc = build_program(meta, plan)

    res = run_bass_kernel_spmd(nc, in_maps, core_ids=list(range(NCORES)),
                               trace=_want_trace)
    out = np.empty((n, 1), np.float32)
    for c in range(NCORES):
        out[c * shard:(c + 1) * shard, 0] = res.results[c]["out"][0, :shard]
    kernel._last_exec_ns = res.exec_time_ns
    return out



# revision 9
# speedup vs baseline: 1.4017x; 1.4017x over previous
"""GCN (2x GCNConv + MLP head) on 8 TRN2 NeuronCores via Bass/Tile.

Scan-based aggregation (v2):
  - nodes sharded by id across 8 cores; weights replicated.
  - Per core, edges with dst in the core's shard split into 8 dst-streams
    (contiguous dst-eighths). Each GpSimd 16-partition group processes one
    stream: ap_gather pulls full 64-feat messages (feat-quad layout
    [16, W, 4] bf16, feat f = p + 16j) from a window table with a
    group-private index stream — 8 streams per call in parallel
    (~3.8 ns/edge vs 8.45 ns/desc for dma_gather).
  - Edges sorted by (window, dst-quarter, dst). Per (window, quarter) one
    gather call; DVE tensor_tensor_scan (state = mask*state + x, fp32
    state) forms running per-dst segment sums; a small ap_gather extracts
    each dst's subrun end; presence-masked adds accumulate the window
    partials into agg[128, sdpad, 4] f32.
  - conv messages m1T/m2T are built by j-split PE matmuls into DRAM
    [16, npad, 4] bf16; window tables load with 8 contiguous DMAs.
    AllGather of h1d shards = halo exchange. MLP head in transposed space.
  - Both convs share one set of idx/mask/extraction arrays (same graph).
"""

import numpy as np
import ml_dtypes

import concourse.bass as bass  # noqa: F401
import concourse.bacc as bacc
import concourse.tile as tile
import concourse.mybir as mybir
from concourse.bass_utils import run_bass_kernel_spmd

F32 = mybir.dt.float32
BF16 = mybir.dt.bfloat16
I16 = mybir.dt.int16

NCORES = 8
NSTREAM = 8          # dst-streams per core == gpsimd 16-partition groups
WIN = 10176          # real nodes per gather window
WPAD = 64            # zero rows appended to each window table
WTBL = WIN + WPAD    # ap_gather num_elems (WTBL*4*2/4 <= 2**15)
D = 4                # feat quad width
NH = 4               # dst-quarter calls per window
SDPAD = 1600         # padded dsts per stream (%16, >= ceil(12500/8))
HALF = SDPAD // NH   # 400 dsts per call
EC = 512             # column chunk for PE phases


def wrapg(a):
    """[8, n] per-group idx streams -> [128, n//16] int16 ap_gather layout."""
    out = np.empty((128, a.shape[1] // 16), np.int16)
    for g in range(NSTREAM):
        out[16 * g:16 * g + 16] = np.ascontiguousarray(a[g].reshape(-1, 16).T)
    return out


def preprocess(n, edge_index):
    src = edge_index[0].astype(np.int64)
    dst = edge_index[1].astype(np.int64)

    deg = np.bincount(dst, minlength=n).astype(np.float64) + 1.0
    dinv = (1.0 / np.sqrt(deg)).astype(np.float32)

    shard = n // NCORES                       # 12500
    sdst = (shard + NSTREAM - 1) // NSTREAM   # 1563 dsts per stream
    nwin = (n + WIN - 1) // WIN               # 10
    npad = nwin * WIN

    loops = np.arange(n, dtype=np.int64)
    srcA = np.concatenate([src, loops])
    dstA = np.concatenate([dst, loops])

    c_of = dstA // shard
    dl = dstA - c_of * shard
    g_of = np.minimum(dl // sdst, NSTREAM - 1)
    sl = dl - g_of * sdst                     # stream-local dst
    w_of = srcA // WIN
    h_of = np.minimum(sl // HALF, NH - 1)

    ncalls = nwin * NH
    key = (((c_of * NSTREAM + g_of) * nwin + w_of) * NH + h_of)
    order = np.lexsort((sl, key))
    srcS, slS = srcA[order], sl[order]

    counts = np.bincount(key, minlength=NCORES * NSTREAM * ncalls)
    starts = np.zeros(len(counts) + 1, np.int64)
    np.cumsum(counts, out=starts[1:])
    cmax = counts.reshape(NCORES, NSTREAM, nwin, NH).max(axis=(0, 1))
    csz = ((cmax + 15) // 16) * 16            # [nwin, NH] shared call sizes
    gtot = int(csz.sum())
    etot = nwin * SDPAD

    cores = []
    for c in range(NCORES):
        gidx = np.full((NSTREAM, gtot), WIN, np.int16)     # pad -> zero row
        mask = np.ones((NSTREAM, gtot), np.float32)
        eidx = np.zeros((NSTREAM, etot), np.int16)
        pres = np.zeros((NSTREAM, etot), np.float32)
        off = 0
        eoff = 0
        for w in range(nwin):
            for h in range(NH):
                cs = int(csz[w, h])
                d0 = h * HALF
                for g in range(NSTREAM):
                    k = ((c * NSTREAM + g) * nwin + w) * NH + h
                    a, b = starts[k], starts[k + 1]
                    cnt = b - a
                    d_loc = slS[a:b]
                    gidx[g, off:off + cnt] = (srcS[a:b] - w * WIN).astype(np.int16)
                    m = np.ones(cnt, np.float32)
                    if cnt:
                        m[0] = 0.0
                        m[1:][d_loc[1:] != d_loc[:-1]] = 0.0
                        lastpos = np.zeros(HALF, np.int64)
                        lastpos[d_loc - d0] = np.arange(cnt)  # last write wins
                        present = np.zeros(HALF, np.float32)
                        present[np.unique(d_loc) - d0] = 1.0
                        eidx[g, eoff:eoff + HALF] = lastpos.astype(np.int16)
                        pres[g, eoff:eoff + HALF] = present
                    mask[g, off:off + cnt] = m
                off += cs
                eoff += HALF
        dinvd = np.zeros((NSTREAM, SDPAD), np.float32)
        base = c * shard
        for g in range(NSTREAM):
            lo = base + g * sdst
            hi = min(base + min((g + 1) * sdst, shard), n)
            if lo < hi:
                dinvd[g, :hi - lo] = dinv[lo:hi]
        cores.append(dict(
            gidx=wrapg(gidx),
            mask=np.repeat(mask, 16, axis=0).astype(ml_dtypes.bfloat16),
            eidx=wrapg(eidx),
            pres=np.repeat(pres, 16, axis=0).astype(ml_dtypes.bfloat16),
            dinvd=np.repeat(dinvd, 16, axis=0),
        ))

    plan = dict(nwin=nwin, npad=npad, shard=shard, sdst=sdst,
                csz=csz, gtot=gtot, etot=etot)
    return dinv, plan, cores


def build_program(plan):
    nwin = plan["nwin"]
    npad = plan["npad"]
    shard = plan["shard"]
    sdst = plan["sdst"]
    csz = plan["csz"]
    gtot = plan["gtot"]
    etot = plan["etot"]
    dpad = SDPAD * NSTREAM
    CSMAX = int(csz.max())

    nc = bacc.Bacc("TRN2", target_bir_lowering=False, debug=False,
                   num_devices=NCORES)

    xt = nc.dram_tensor("xt", [128, npad], BF16, kind="ExternalInput")
    gidx_d = nc.dram_tensor("gidx", [128, gtot // 16], I16, kind="ExternalInput")
    mask_d = nc.dram_tensor("mask", [128, gtot], BF16, kind="ExternalInput")
    eidx_d = nc.dram_tensor("eidx", [128, etot // 16], I16, kind="ExternalInput")
    pres_d = nc.dram_tensor("pres", [128, etot], BF16, kind="ExternalInput")
    dinvd_d = nc.dram_tensor("dinvd", [128, SDPAD], F32, kind="ExternalInput")
    w1_d = nc.dram_tensor("w1", [128, 64], BF16, kind="ExternalInput")
    w2_d = nc.dram_tensor("w2", [64, 64], BF16, kind="ExternalInput")
    lw1_d = nc.dram_tensor("lw1", [64, 64], F32, kind="ExternalInput")
    lw2_d = nc.dram_tensor("lw2", [64, 32], F32, kind="ExternalInput")
    lw3_d = nc.dram_tensor("lw3", [32, 1], F32, kind="ExternalInput")
    b1q_d = nc.dram_tensor("b1q", [128, 4], F32, kind="ExternalInput")
    b2q_d = nc.dram_tensor("b2q", [128, 4], F32, kind="ExternalInput")
    lb1_d = nc.dram_tensor("lb1", [64, 1], F32, kind="ExternalInput")
    lb2_d = nc.dram_tensor("lb2", [32, 1], F32, kind="ExternalInput")
    lb3_d = nc.dram_tensor("lb3", [1, 1], F32, kind="ExternalInput")
    out_d = nc.dram_tensor("out", [1, dpad], F32, kind="ExternalOutput")

    with tile.TileContext(nc) as tc:
        with (
            tc.tile_pool(name="const", bufs=1) as cpool,
            tc.tile_pool(name="tblp", bufs=1) as tblp,
            tc.tile_pool(name="chk", bufs=2) as chk,
            tc.tile_pool(name="meta", bufs=2) as mpool,
            tc.tile_pool(name="ext", bufs=1) as epool,
            tc.tile_pool(name="epi", bufs=1) as epip,
            tc.tile_pool(name="work", bufs=2) as pool,
            tc.tile_pool(name="psA", bufs=2, space="PSUM") as psA,
            tc.tile_pool(name="psM", bufs=2, space="PSUM") as psM,
            tc.tile_pool(name="dram", bufs=1, space="DRAM") as dram,
        ):
            def load_const(dram_t, shape, dtype, tag):
                t = cpool.tile(shape, dtype, tag=tag)
                nc.sync.dma_start(t[:], dram_t[:])
                return t

            w1_t = load_const(w1_d, [128, 64], BF16, "w1")
            w2_t = load_const(w2_d, [64, 64], BF16, "w2")
            lw1_t = load_const(lw1_d, [64, 64], F32, "lw1")
            lw2_t = load_const(lw2_d, [64, 32], F32, "lw2")
            lw3_t = load_const(lw3_d, [32, 1], F32, "lw3")
            b1q_t = load_const(b1q_d, [128, 4], F32, "b1q")
            b2q_t = load_const(b2q_d, [128, 4], F32, "b2q")
            lb1_t = load_const(lb1_d, [64, 1], F32, "lb1")
            lb2_t = load_const(lb2_d, [32, 1], F32, "lb2")
            lb3_t = load_const(lb3_d, [1, 1], F32, "lb3")
            dinvd_t = load_const(dinvd_d, [128, SDPAD], F32, "dinvd")

            m1T = dram.tile([16, npad, D], BF16)
            m2T = dram.tile([16, npad, D], BF16)
            h1db = dram.tile([16, 4, dpad], BF16)
            h2b = dram.tile([16, 4, dpad], F32)
            ag_out = dram.tile([NCORES * 64, dpad], BF16, addr_space="Shared")

            tbl = tblp.tile([128, WTBL, D], BF16, tag="tbl")
            nc.vector.memset(tbl[:, WIN:, :], 0.0)
            agg = tblp.tile([128, SDPAD, D], F32, tag="agg")

            def emit_msgs(lhsT_full, rhs_cols_cb, ncols, outT):
                """outT[:, o:o+cw, :] = quad-split matmul of rhs columns."""
                nchunk = (ncols + EC - 1) // EC
                for t in range(nchunk):
                    o = t * EC
                    cw = min(EC, ncols - o)
                    rhs = rhs_cols_cb(o, cw)
                    stg = pool.tile([16, EC, D], BF16, tag="stg")
                    for j in range(D):
                        ps = psA.tile([16, EC], F32, tag="psa")
                        nc.tensor.matmul(ps[:, :cw],
                                         lhsT=lhsT_full[:, 16 * j:16 * j + 16],
                                         rhs=rhs, start=True, stop=True)
                        nc.vector.tensor_copy(stg[:, :cw, j], ps[:, :cw])
                    nc.sync.dma_start(outT[:, o:o + cw, :], stg[:, :cw, :])

            # --- phase A: m1T = W1^T xt (quad layout) ---
            def xt_cols(o, cw):
                st = pool.tile([128, EC], BF16, tag="xt")
                nc.sync.dma_start(st[:, :cw], xt[:, o:o + cw])
                return st[:, :cw]

            emit_msgs(w1_t, xt_cols, npad, m1T)

            def conv(msgT, bq_t, out_cb):
                nc.vector.memset(agg[:], 0.0)
                goff = 0
                eoff = 0
                for w in range(nwin):
                    for g in range(NSTREAM):
                        nc.sync.dma_start(
                            tbl[16 * g:16 * g + 16, :WIN, :],
                            msgT[:, w * WIN:(w + 1) * WIN, :])
                    for h in range(NH):
                        cs = int(csz[w, h])
                        it = mpool.tile([128, CSMAX // 16], I16, tag="gi")
                        nc.sync.dma_start(it[:, :cs // 16],
                                          gidx_d[:, goff // 16:(goff + cs) // 16])
                        mt = mpool.tile([128, CSMAX], BF16, tag="mk")
                        nc.sync.dma_start(mt[:, :cs], mask_d[:, goff:goff + cs])
                        gch = chk.tile([128, CSMAX, D], BF16, tag="gch")
                        nc.gpsimd.ap_gather(
                            gch[:, :cs, :], tbl[:], it[:, :cs // 16],
                            channels=128, num_elems=WTBL, d=D, num_idxs=cs)
                        for j in range(D):
                            nc.vector.tensor_tensor_scan(
                                gch[:, :cs, j], mt[:, :cs], gch[:, :cs, j],
                                0.0, op0=mybir.AluOpType.mult,
                                op1=mybir.AluOpType.add)
                        et = mpool.tile([128, HALF // 16], I16, tag="ei")
                        nc.sync.dma_start(
                            et[:], eidx_d[:, eoff // 16:(eoff + HALF) // 16])
                        pt = mpool.tile([128, HALF], BF16, tag="pr")
                        nc.sync.dma_start(pt[:], pres_d[:, eoff:eoff + HALF])
                        ex = epool.tile([128, HALF, D], BF16, tag="ex")
                        nc.gpsimd.ap_gather(
                            ex[:], gch[:, :cs, :], et[:],
                            channels=128, num_elems=cs, d=D, num_idxs=HALF)
                        exm = epool.tile([128, HALF, D], F32, tag="exm")
                        nc.vector.tensor_tensor(
                            exm[:], ex[:],
                            pt[:].unsqueeze(2).broadcast_to([128, HALF, D]),
                            op=mybir.AluOpType.mult)
                        d0 = h * HALF
                        nc.vector.tensor_tensor(
                            agg[:, d0:d0 + HALF, :], agg[:, d0:d0 + HALF, :],
                            exm[:], op=mybir.AluOpType.add)
                        goff += cs
                        eoff += HALF
                out_cb()

            # --- conv1 epilogue: h1d = dinv*(dinv*agg + b1) -> h1db ---
            def conv1_out():
                for h in range(NH):
                    d0 = h * HALF
                    e1 = epip.tile([128, HALF, D], F32, tag="e1")
                    nc.vector.tensor_tensor(
                        e1[:], agg[:, d0:d0 + HALF, :],
                        dinvd_t[:, d0:d0 + HALF].unsqueeze(2)
                        .broadcast_to([128, HALF, D]),
                        op=mybir.AluOpType.mult)
                    nc.vector.tensor_tensor(
                        e1[:], e1[:],
                        b1q_t[:].unsqueeze(1).broadcast_to([128, HALF, D]),
                        op=mybir.AluOpType.add)
                    e2 = epip.tile([128, D, HALF], BF16, tag="e2")
                    nc.vector.tensor_tensor(
                        e2[:], e1[:].transpose([0, 2, 1]),
                        dinvd_t[:, d0:d0 + HALF].unsqueeze(1)
                        .broadcast_to([128, D, HALF]),
                        op=mybir.AluOpType.mult)
                    for g in range(NSTREAM):
                        nc.sync.dma_start(
                            h1db[:, :, g * SDPAD + d0:g * SDPAD + d0 + HALF],
                            e2[16 * g:16 * g + 16, :, :].opt())

            conv(m1T, b1q_t, conv1_out)

            nc.gpsimd.collective_compute(
                "AllGather", mybir.AluOpType.bypass,
                ins=[h1db[:].opt()],
                outs=[ag_out[:].opt()],
                replica_groups=[list(range(NCORES))],
            )

            # --- phase C: m2T = W2^T h1dT (per core, per stream segment) ---
            for c in range(NCORES):
                for g in range(NSTREAM):
                    cnt = min(sdst, shard - g * sdst)
                    node0 = c * shard + g * sdst

                    def ag_cols(o, cw, c=c, g=g):
                        st = pool.tile([64, EC], BF16, tag="agc")
                        nc.sync.dma_start(
                            st[:, :cw],
                            ag_out[c * 64:(c + 1) * 64,
                                   g * SDPAD + o:g * SDPAD + o + cw])
                        return st[:, :cw]

                    emit_msgs(w2_t, ag_cols, cnt,
                              m2T[:, node0:node0 + cnt, :])

            # --- conv2 epilogue: h2 = dinv*agg + b2 -> h2b (f32) ---
            def conv2_out():
                for h in range(NH):
                    d0 = h * HALF
                    e1 = epip.tile([128, HALF, D], F32, tag="e1")
                    nc.vector.tensor_tensor(
                        e1[:], agg[:, d0:d0 + HALF, :],
                        dinvd_t[:, d0:d0 + HALF].unsqueeze(2)
                        .broadcast_to([128, HALF, D]),
                        op=mybir.AluOpType.mult)
                    e2f = epip.tile([128, D, HALF], F32, tag="e2f")
                    nc.vector.tensor_tensor(
                        e2f[:], e1[:].transpose([0, 2, 1]),
                        b2q_t[:].unsqueeze(2).broadcast_to([128, D, HALF]),
                        op=mybir.AluOpType.add)
                    for g in range(NSTREAM):
                        nc.sync.dma_start(
                            h2b[:, :, g * SDPAD + d0:g * SDPAD + d0 + HALF],
                            e2f[16 * g:16 * g + 16, :, :].opt())

            conv(m2T, b2q_t, conv2_out)

            # --- MLP head (transposed space, h2 streamed from DRAM) ---
            for o in range(0, dpad, EC):
                w_ = min(EC, dpad - o)
                h2c = pool.tile([64, EC], F32, tag="h2c")
                nc.sync.dma_start(h2c[:, :w_], h2b[:, :, o:o + w_])
                p1 = psM.tile([64, EC], F32, tag="mm1")
                nc.tensor.matmul(p1[:, :w_], lhsT=lw1_t[:],
                                 rhs=h2c[:, :w_], start=True, stop=True)
                z1 = pool.tile([64, EC], F32, tag="z1")
                nc.scalar.activation(z1[:, :w_], p1[:, :w_],
                                     mybir.ActivationFunctionType.Relu,
                                     bias=lb1_t[:])
                p2 = psM.tile([32, EC], F32, tag="mm2")
                nc.tensor.matmul(p2[:, :w_], lhsT=lw2_t[:], rhs=z1[:, :w_],
                                 start=True, stop=True)
                z2 = pool.tile([32, EC], F32, tag="z2")
                nc.scalar.activation(z2[:, :w_], p2[:, :w_],
                                     mybir.ActivationFunctionType.Relu,
                                     bias=lb2_t[:])
                p3 = psM.tile([1, EC], F32, tag="mm3")
                nc.tensor.matmul(p3[:, :w_], lhsT=lw3_t[:], rhs=z2[:, :w_],
                                 start=True, stop=True)
                z3 = pool.tile([1, EC], F32, tag="z3")
                nc.vector.tensor_tensor(z3[:, :w_], p3[:, :w_],
                                        lb3_t[:].broadcast_to([1, w_]),
                                        op=mybir.AluOpType.add)
                nc.sync.dma_start(out_d[:, o:o + w_], z3[:, :w_])

    nc.compile()
    return nc


def kernel(x, edge_index, W1, b1, W2, b2, lw1, lb1, lw2, lb2, lw3, lb3,
           _want_trace=False):
    x = np.asarray(x, np.float32)
    edge_index = np.asarray(edge_index)
    n = x.shape[0]

    dinv, plan, cores = preprocess(n, edge_index)
    shard, sdst, npad = plan["shard"], plan["sdst"], plan["npad"]

    xt = np.zeros((128, npad), ml_dtypes.bfloat16)
    xt[:, :n] = (x * dinv[:, None]).T.astype(ml_dtypes.bfloat16)

    # h1db/h2b row r = 4p+j holds feat p+16j -> permute consumer weight rows
    perm = np.array([(r // 4) + 16 * (r % 4) for r in range(64)])

    def quadb(b):
        # bias for [16g+p, j] = b[p + 16j]
        q = np.asarray(b, np.float32).reshape(4, 16).T  # [p, j]
        return np.ascontiguousarray(np.tile(q, (8, 1)))

    in_maps = []
    for c in range(NCORES):
        in_maps.append({
            "xt": xt,
            "gidx": cores[c]["gidx"], "mask": cores[c]["mask"],
            "eidx": cores[c]["eidx"], "pres": cores[c]["pres"],
            "dinvd": cores[c]["dinvd"],
            "w1": np.asarray(W1, np.float32).astype(ml_dtypes.bfloat16),
            "w2": np.ascontiguousarray(
                np.asarray(W2, np.float32)[perm]).astype(ml_dtypes.bfloat16),
            "lw1": np.ascontiguousarray(np.asarray(lw1, np.float32)[perm]),
            "lw2": np.ascontiguousarray(np.asarray(lw2, np.float32)),
            "lw3": np.ascontiguousarray(np.asarray(lw3, np.float32)),
            "b1q": quadb(b1), "b2q": quadb(b2),
            "lb1": np.asarray(lb1, np.float32).reshape(-1, 1),
            "lb2": np.asarray(lb2, np.float32).reshape(-1, 1),
            "lb3": np.asarray(lb3, np.float32).reshape(-1, 1),
        })

    nc = build_program(plan)
    res = run_bass_kernel_spmd(nc, in_maps, core_ids=list(range(NCORES)),
                               trace=_want_trace)
    out = np.empty((n, 1), np.float32)
    for c in range(NCORES):
        o = res.results[c]["out"][0]
        v = o.reshape(NSTREAM, SDPAD)[:, :sdst].reshape(-1)[:shard]
        out[c * shard:(c + 1) * shard, 0] = v
    kernel._last_exec_ns = res.exec_time_ns
    return out
